# revision 33
# baseline (speedup 1.0000x reference)
"""BertCrossAttention Trainium2 Bass kernel.

Data-parallel over batch: 8 batch elements -> 8 NeuronCores, one SPMD NEFF,
no collectives.  kernel(**inputs) takes the full unsharded inputs and
returns (out, probs) exactly like the reference.

Per-core computation (one batch element):
  q = query @ Wq.T + bq          [512, 768]
  k = key   @ Wk.T + bk          [1024, 768]
  v = value @ Wv.T + bv          [1024, 768]
  per head h (12 heads, D=64):
    sT = k_h @ q_h.T / 8 + mask  [1024, 512]   (scores transposed)
    p  = exp(sT); row sums come for free from a ones-column in v
    ctxT_h = v_h.T @ p / rowsum  [64, 512]
  h = ctx @ Wo.T + bo + query; LayerNorm(h) -> out [512, 768]

Design notes:
- Everything is kept feature-major ("transposed") so no on-device
  transposes are needed anywhere.  The host pre-transposes activations and
  weights while sharding.
- Biases are folded exactly: bq/bk as per-partition adds fused into the
  projection PSUM evictions; bv/bo into the residual
  q_res = query + bo + bv @ Wo.T (exact because softmax rows sum to 1).
- The whole matmul path runs in float16 (10 mantissa bits; measured probs
  max rel err ~7e-4 vs the fp32 reference).  The residual + LayerNorm path
  stays fp32.  fp32/f32r matmuls stream at 2 cycles/row on TRN2 PE; fp16
  streams at 1 and enables fast weight load.
- Scores are computed transposed per head pair: K=64 matmuls for heads
  2j/2j+1 land in row groups 0-1/2-3 of the PE and run concurrently; both
  heads share one 2-bank PSUM tile so a single exp ACTIVATE per m-tile
  evicts them (per-partition mask bias + 1/8 scale fused).
- The softmax denominator rides the ctx matmul: v is augmented with a
  ones column per head, so PSUM row 64 of ctx is the row sum.  Normalize =
  DVE reciprocal + Pool partition_broadcast + one strided fp16 multiply.
- probs are produced transposed [H, Lk, Lq] as fp16 and DMAd on the SWDGE
  queue; the host transposes back and upcasts while gathering.
"""

import numpy as np
from contextlib import ExitStack

import concourse.bass as bass
import concourse.mybir as mybir
import concourse.tile as tile
from concourse import bacc
from concourse.bass_utils import run_bass_kernel_spmd

F32 = mybir.dt.float32
F32R = mybir.dt.float32r
F16 = mybir.dt.float16
AF = mybir.ActivationFunctionType
ALU = mybir.AluOpType

HIDDEN = 768
HEADS = 12
D = 64
LQ = 512
LK = 1024
B = 8
P = 128
KT = HIDDEN // P          # 6 k-tiles of 128 (plus 1 aug row for q/k proj)
QT = LQ // P              # 4 l-tiles
MT = LK // P              # 8 m-tiles
LN_EPS = 1e-12


def build_nc():
    nc = bacc.Bacc("TRN2", target_bir_lowering=False, debug=False)

    # ---- DRAM I/O ------------------------------------------------------
    qT = nc.dram_tensor("qT", [HIDDEN, LQ], F16, kind="ExternalInput")
    kT = nc.dram_tensor("kT", [HIDDEN, LK], F16, kind="ExternalInput")
    vT = nc.dram_tensor("vT", [HIDDEN, LK], F16, kind="ExternalInput")
    wqT = nc.dram_tensor("wqT", [HIDDEN, HIDDEN], F16, kind="ExternalInput")
    wkT = nc.dram_tensor("wkT", [HIDDEN, HIDDEN], F16, kind="ExternalInput")
    bq2 = nc.dram_tensor("bq2", [P, KT], F32, kind="ExternalInput")
    bk2 = nc.dram_tensor("bk2", [P, KT], F32, kind="ExternalInput")
    wvT = nc.dram_tensor("wvT", [HIDDEN, HIDDEN], F16, kind="ExternalInput")
    woT = nc.dram_tensor("woT", [HIDDEN, HIDDEN], F16, kind="ExternalInput")
    q_res = nc.dram_tensor("q_res", [LQ, HIDDEN], F32, kind="ExternalInput")
    mask2 = nc.dram_tensor("mask2", [P, MT], F32, kind="ExternalInput")
    gamma = nc.dram_tensor("gamma", [HIDDEN], F32, kind="ExternalInput")
    beta = nc.dram_tensor("beta", [HIDDEN], F32, kind="ExternalInput")

    out = nc.dram_tensor("out", [LQ, HIDDEN], F32, kind="ExternalOutput")
    probsT = nc.dram_tensor("probsT", [HEADS, LK, LQ], F16, kind="ExternalOutput")

    with tile.TileContext(nc) as tc, ExitStack() as top:
        persist = top.enter_context(tc.tile_pool(name="persist", bufs=1))

        # persistent SBUF tensors
        qpT = [persist.tile([P, LQ], F16, name=f"qpT{j}") for j in range(KT)]
        kpT = [persist.tile([P, LK], F16, name=f"kpT{j}") for j in range(KT)]
        # v augmented with a per-head ones column: [m, 12*(64+1)]
        v_aug = [persist.tile([P, HEADS * (D + 1)], F16, name=f"vaug{m}")
                 for m in range(MT)]
        ctxT = [persist.tile([P, LQ], F16, name=f"ctxT{j}") for j in range(KT)]
        mask_sb = persist.tile([P, MT], F32, name="mask_sb")
        nc.sync.dma_start(out=mask_sb[:], in_=mask2[:])
        ones_f32 = persist.tile([P, HEADS, 1], F32, name="ones_f32")
        nc.vector.memset(ones_f32[:], 1.0)

        # ---- phase 1: QKV projections ---------------------------------
        with ExitStack() as proj:
            ppool = proj.enter_context(tc.tile_pool(name="proj_psum", bufs=2,
                                                    space="PSUM"))
            pq = proj.enter_context(tc.tile_pool(name="pq", bufs=1))
            wq_sb = [pq.tile([P, HIDDEN], F16, name=f"wq{j}") for j in range(KT)]
            qT_sb = [pq.tile([P, LQ], F16, name=f"qTs{j}") for j in range(KT)]
            bq_sb = pq.tile([P, KT], F32, name="bq_sb")
            bk_sb = pq.tile([P, KT], F32, name="bk_sb")
            nc.sync.dma_start(out=bq_sb[:], in_=bq2[:])
            nc.sync.dma_start(out=bk_sb[:], in_=bk2[:])
            for j in range(KT):
                nc.sync.dma_start(out=wq_sb[j][:], in_=wqT[j * P:(j + 1) * P, :])
                nc.sync.dma_start(out=qT_sb[j][:], in_=qT[j * P:(j + 1) * P, :])

            pk = proj.enter_context(tc.tile_pool(name="pk", bufs=1))
            wk_sb = [pk.tile([P, HIDDEN], F16, name=f"wk{j}") for j in range(KT)]
            kT_sb = [pk.tile([P, LK], F16, name=f"kTs{j}") for j in range(KT)]
            for j in range(KT):
                nc.sync.dma_start(out=wk_sb[j][:], in_=wkT[j * P:(j + 1) * P, :])
                nc.sync.dma_start(out=kT_sb[j][:], in_=kT[j * P:(j + 1) * P, :])

            pv = proj.enter_context(tc.tile_pool(name="pv", bufs=1))
            wv_sb = [pv.tile([P, HIDDEN], F16, name=f"wv{j}") for j in range(KT)]
            vT_sb = [pv.tile([P, LK], F16, name=f"vTs{j}") for j in range(KT)]

            for j in range(KT):
                nc.sync.dma_start(out=wv_sb[j][:], in_=wvT[j * P:(j + 1) * P, :])
                nc.sync.dma_start(out=vT_sb[j][:], in_=vT[j * P:(j + 1) * P, :])

            # q projection: qpT[io] [128, 512] = sum_j wq[j][:,io].T @ qT[j]
            for io in range(KT):
                ps = ppool.tile([P, LQ], F32, name="ps_p")
                for j in range(KT):
                    nc.tensor.matmul(ps[:], lhsT=wq_sb[j][:, io * P:(io + 1) * P],
                                     rhs=qT_sb[j][:], start=(j == 0),
                                     stop=(j == KT - 1))
                nc.vector.tensor_scalar_add(qpT[io][:], ps[:],
                                            bq_sb[:, io:io + 1])

            # k projection: kpT[io] [128, 1024], two 512-wide chunks
            for io in range(KT):
                for c in range(2):
                    cs = slice(c * 512, (c + 1) * 512)
                    ps = ppool.tile([P, 512], F32, name="ps_p")
                    for j in range(KT):
                        nc.tensor.matmul(ps[:], lhsT=wk_sb[j][:, io * P:(io + 1) * P],
                                         rhs=kT_sb[j][:, cs], start=(j == 0),
                                         stop=(j == KT - 1))
                    nc.vector.tensor_scalar_add(kpT[io][:, cs], ps[:],
                                                bk_sb[:, io:io + 1])

            # v projection -> v_aug strided per-head columns.
            # psum [m-tile 128, 384] covers heads 6c..6c+5 (6*64 outputs)
            va_view = [v.rearrange("p (h e) -> p h e", h=HEADS) for v in v_aug]
            for m in range(MT):
                for c in range(2):
                    ps = ppool.tile([P, 384], F32, name="ps_p")
                    for j in range(KT):
                        nc.tensor.matmul(
                            ps[:],
                            lhsT=vT_sb[j][:, m * P:(m + 1) * P],
                            rhs=wv_sb[j][:, c * 384:(c + 1) * 384],
                            start=(j == 0), stop=(j == KT - 1))
                    nc.vector.tensor_copy(
                        va_view[m][:, 6 * c:6 * c + 6, 0:D],
                        ps[:].rearrange("p (h e) -> p h e", h=6))
                # ones columns for the row-sum trick (walrus rejects
                # non-f32 memset here, so copy from an f32 staging tile)
                nc.vector.tensor_copy(va_view[m][:, :, D:D + 1], ones_f32[:])

        # output-projection weights: load during attention (sync queue idle)
        wop = top.enter_context(tc.tile_pool(name="wop", bufs=1))
        wo_sb = [wop.tile([P, HIDDEN], F16, name=f"wo{j}") for j in range(KT)]
        for j in range(KT):
            nc.sync.dma_start(out=wo_sb[j][:], in_=woT[j * P:(j + 1) * P, :])

        # ---- phase 2: attention ---------------------------------------
        attn = ExitStack()
        # scores for a head pair share one 2-bank psum tile -> one exp
        # instruction per m-tile with an exact per-partition mask bias
        sc_psum = attn.enter_context(tc.tile_pool(name="sc_psum", bufs=2,
                                                  space="PSUM"))
        cx_psum = attn.enter_context(tc.tile_pool(name="cx_psum", bufs=2,
                                                  space="PSUM"))
        pt_pool = attn.enter_context(tc.tile_pool(name="pt_pool", bufs=4))
        nrm_pool = attn.enter_context(tc.tile_pool(name="nrm_pool", bufs=2))

        pT_pairs = {}  # pair -> [128, MT*1024] tile (m-tile blocks of 1024:
                       #          [head even 512 | head odd 512])

        def emit_scores(pair):
            """scores + exp for heads (2*pair, 2*pair+1)."""
            j = pair
            pTp = pt_pool.tile([P, MT * 1024], F16, name="pT", tag="pT")
            pT_pairs[pair] = pTp
            for m in range(MT):
                ps = sc_psum.tile([P, 1024], F32, name="ps_s")
                for sub in range(2):
                    rows = slice(sub * D, (sub + 1) * D)
                    nc.tensor.matmul(ps[:, sub * 512:(sub + 1) * 512],
                                     lhsT=kpT[j][rows, m * P:(m + 1) * P],
                                     rhs=qpT[j][rows, :], start=True, stop=True)
                # p = exp(s/8 + mask): one eviction for both heads
                nc.scalar.activation(
                    pTp[:, m * 1024:(m + 1) * 1024], ps[:],
                    AF.Exp, bias=mask_sb[:, m:m + 1], scale=0.125)

        def emit_ctx(pair):
            """ctx matmul + softmax normalize + probs DMA for a done pair."""
            pTp = pT_pairs.pop(pair)
            cps_pair = []
            for sub in range(2):
                h = 2 * pair + sub
                cps = cx_psum.tile([D + 1, LQ], F32, name="ps_c")
                cps_pair.append(cps)
                for m in range(MT):
                    nc.tensor.matmul(
                        cps[:],
                        lhsT=v_aug[m][:, h * (D + 1):(h + 1) * (D + 1)],
                        rhs=pTp[:, m * 1024 + sub * 512:
                                m * 1024 + (sub + 1) * 512],
                        start=(m == 0), stop=(m == MT - 1))
                # evict ctx rows early to free psum (DVE, casts to fp16)
                prow = slice((h % 2) * D, (h % 2) * D + D)
                nc.vector.tensor_copy(ctxT[h // 2][prow, :], cps[0:D, :])
            # pair reciprocal of rowsums (psum row 64)
            recips = []
            for sub in range(2):
                recip = nrm_pool.tile([1, LQ], F16, name="recip")
                with nc.allow_low_precision(reason="fp16 softmax recip"):
                    nc.vector.reciprocal(recip[:], cps_pair[sub][D:D + 1, :])
                recips.append(recip)
            for sub in range(2):
                h = 2 * pair + sub
                prow = slice((h % 2) * D, (h % 2) * D + D)
                # broadcast recip across partitions (Pool)
                recip_b = nrm_pool.tile([P, LQ], F16, name="recip_b")
                nc.gpsimd.partition_broadcast(recip_b[:], recips[sub][:])
                # normalize ctx rows and probs (DVE, fp16)
                nc.vector.tensor_mul(ctxT[h // 2][prow, :],
                                     ctxT[h // 2][prow, :], recip_b[prow, :])
                ph_v = bass.AP(tensor=pTp.tensor,
                               offset=pTp.offset + sub * 512,
                               ap=[list(pTp.ap[0]), [1024, MT], [1, 512]])
                rb_rep = bass.AP(tensor=recip_b.tensor, offset=recip_b.offset,
                                 ap=[list(recip_b.ap[0]), [0, MT],
                                     list(recip_b.ap[1])])
                nc.vector.tensor_mul(ph_v, ph_v, rb_rep)
                # probs out on the SWDGE queue (keeps sync HWDGE FIFO clear)
                dram = probsT[h].rearrange("(m p) l -> p m l", p=P)
                nc.gpsimd.dma_start(out=dram, in_=ph_v)

        for pair in range(6):
            emit_scores(pair)
            if pair >= 2:
                emit_ctx(pair - 2)
        emit_ctx(4)
        emit_ctx(5)

        # tail pools open inside the attention scope: their PSUM banks and
        # SBUF are disjoint from the attention pools, so the output dense
        # can start while the last softmax chains drain
        late = attn.enter_context(tc.tile_pool(name="late", bufs=1))
        qres_sb = [late.tile([P, HIDDEN], F32, name=f"qres{t}") for t in range(QT)]
        for t in range(QT):
            nc.sync.dma_start(out=qres_sb[t][:], in_=q_res[t * P:(t + 1) * P, :])
        gamma_sb = late.tile([P, HIDDEN], F32, name="gamma_sb")
        beta_sb = late.tile([P, HIDDEN], F32, name="beta_sb")
        nc.gpsimd.dma_start(out=gamma_sb[:], in_=gamma.ap().partition_broadcast(P))
        nc.gpsimd.dma_start(out=beta_sb[:], in_=beta.ap().partition_broadcast(P))

        # ---- phase 3: output dense + residual + LayerNorm -------------
        o_psum = attn.enter_context(tc.tile_pool(name="o_psum", bufs=2,
                                                 space="PSUM"))
        h_pool = attn.enter_context(tc.tile_pool(name="h_pool", bufs=2))
        st_pool = attn.enter_context(tc.tile_pool(name="st_pool", bufs=4))
        eps_sb = late.tile([P, 1], F32, name="eps_sb")
        nc.vector.memset(eps_sb[:], LN_EPS)

        for t in range(QT):
            ls = slice(t * P, (t + 1) * P)
            h_sb = h_pool.tile([P, HIDDEN], F32, name="h_sb")
            for c in range(2):
                cs = slice(c * 384, (c + 1) * 384)
                ps = o_psum.tile([P, 384], F32, name="ps_o")
                for j in range(KT):
                    nc.tensor.matmul(ps[:], lhsT=ctxT[j][:, ls],
                                     rhs=wo_sb[j][:, cs],
                                     start=(j == 0), stop=(j == KT - 1))
                # h = ctx@WoT + (query + bo + bv@WoT)
                nc.vector.scalar_tensor_tensor(
                    out=h_sb[:, cs], in0=ps[:], scalar=1.0,
                    in1=qres_sb[t][:, cs], op0=ALU.mult, op1=ALU.add)
            # LayerNorm over free dim (768) via bn_stats subgroups of 256
            stats = st_pool.tile([P, 3, 6], F32, name="stats")
            hv = h_sb.rearrange("p (s d) -> p s d", s=3)
            for s in range(3):
                nc.vector.bn_stats(out=stats[:, s, :], in_=hv[:, s, :])
            mv = st_pool.tile([P, 2], F32, name="mv")
            nc.vector.bn_aggr(out=mv[:], in_=stats[:])
            # rstd = 1/sqrt(var + eps)
            rstd = st_pool.tile([P, 1], F32, name="rstd")
            nc.scalar.activation(rstd[:], mv[:, 1:2], AF.Sqrt, bias=eps_sb[:])
            nc.vector.reciprocal(rstd[:], rstd[:])
            # out = (h - mu) * rstd * gamma + beta
            nc.vector.tensor_scalar_sub(h_sb[:], h_sb[:], mv[:, 0:1])
            o_sb = h_pool.tile([P, HIDDEN], F32, name="o_sb")
            nc.vector.scalar_tensor_tensor(
                out=o_sb[:], in0=h_sb[:], scalar=rstd[:],
                in1=gamma_sb[:], op0=ALU.mult, op1=ALU.mult)
            nc.vector.tensor_add(o_sb[:], o_sb[:], beta_sb[:])
            nc.sync.dma_start(out=out[ls, :], in_=o_sb[:])

        attn.close()

    nc.compile()
    return nc


_NC_CACHE = {}


def _get_nc():
    if "nc" not in _NC_CACHE:
        _NC_CACHE["nc"] = build_nc()
    return _NC_CACHE["nc"]


def _prep_core(query_b, key_b, value_b, mask_b, WqT, WkT, WvT, WoT,
               q_res_b, bq2, bk2, gamma, beta):
    qT = np.ascontiguousarray(query_b.T.astype(np.float16))
    kT = np.ascontiguousarray(key_b.T.astype(np.float16))
    vT = np.ascontiguousarray(value_b.T.astype(np.float16))
    mask2 = np.ascontiguousarray(mask_b.reshape(MT, P).T)
    return {
        "qT": qT, "kT": kT, "vT": vT,
        "wqT": WqT, "wkT": WkT, "wvT": WvT, "woT": WoT,
        "q_res": q_res_b, "bq2": bq2, "bk2": bk2, "mask2": mask2,
        "gamma": gamma, "beta": beta,
    }


def _run(inputs, trace=False):
    query = np.asarray(inputs["query"], np.float32)
    key = np.asarray(inputs["key"], np.float32)
    value = np.asarray(inputs["value"], np.float32)
    mask = np.asarray(inputs["attention_mask"], np.float32)
    Wq = np.asarray(inputs["Wq"], np.float32)
    bq = np.asarray(inputs["bq"], np.float32)
    Wk = np.asarray(inputs["Wk"], np.float32)
    bk = np.asarray(inputs["bk"], np.float32)
    Wv = np.asarray(inputs["Wv"], np.float32)
    bv = np.asarray(inputs["bv"], np.float32)
    Wo = np.asarray(inputs["Wo"], np.float32)
    bo = np.asarray(inputs["bo"], np.float32)
    gamma = np.asarray(inputs["ln_gamma"], np.float32)
    beta = np.asarray(inputs["ln_beta"], np.float32)

    WqT = np.ascontiguousarray(Wq.T.astype(np.float16))
    WkT = np.ascontiguousarray(Wk.T.astype(np.float16))
    WvT = np.ascontiguousarray(Wv.T.astype(np.float16))
    WoT = np.ascontiguousarray(Wo.T.astype(np.float16))
    bq2 = np.ascontiguousarray(bq.reshape(KT, P).T)
    bk2 = np.ascontiguousarray(bk.reshape(KT, P).T)
    res_const = (bo + bv @ Wo.T)[None, :]

    nc = _get_nc()
    in_maps = []
    for b in range(B):
        q_res_b = np.ascontiguousarray(query[b] + res_const)
        in_maps.append(_prep_core(query[b], key[b], value[b], mask[b, 0, 0],
                                  WqT, WkT, WvT, WoT, q_res_b, bq2, bk2,
                                  gamma, beta))
    res = run_bass_kernel_spmd(nc, in_maps, list(range(B)), trace=trace)
    out = np.stack([res.results[c]["out"] for c in range(B)])
    probs = np.stack([res.results[c]["probsT"].astype(np.float32).transpose(0, 2, 1)
                      for c in range(B)])
    return out, probs, res


def kernel(**inputs):
    out, probs, _ = _run(inputs)
    return out, probs


# revision 34
# speedup vs baseline: 1.0085x; 1.0085x over previous
"""BertCrossAttention Trainium2 Bass kernel.

Data-parallel over batch: 8 batch elements -> 8 NeuronCores, one SPMD NEFF,
no collectives.  kernel(**inputs) takes the full unsharded inputs and
returns (out, probs) exactly like the reference.

Per-core computation (one batch element):
  q = query @ Wq.T + bq          [512, 768]
  k = key   @ Wk.T + bk          [1024, 768]
  v = value @ Wv.T + bv          [1024, 768]
  per head h (12 heads, D=64):
    sT = k_h @ q_h.T / 8 + mask  [1024, 512]   (scores transposed)
    p  = exp(sT); row sums come for free from a ones-column in v
    ctxT_h = v_h.T @ p / rowsum  [64, 512]
  h = ctx @ Wo.T + bo + query; LayerNorm(h) -> out [512, 768]

Design notes:
- Everything is kept feature-major ("transposed") so no on-device
  transposes are needed anywhere.  The host pre-transposes activations and
  weights while sharding.
- Biases are folded exactly: bq/bk as per-partition adds fused into the
  projection PSUM evictions; bv/bo into the residual
  q_res = query + bo + bv @ Wo.T (exact because softmax rows sum to 1).
- The whole matmul path runs in float16 (10 mantissa bits; measured probs
  max rel err ~7e-4 vs the fp32 reference).  The residual + LayerNorm path
  stays fp32.  fp32/f32r matmuls stream at 2 cycles/row on TRN2 PE; fp16
  streams at 1 and enables fast weight load.
- Scores are computed transposed per head pair: K=64 matmuls for heads
  2j/2j+1 land in row groups 0-1/2-3 of the PE and run concurrently; both
  heads share one 2-bank PSUM tile so a single exp ACTIVATE per m-tile
  evicts them (per-partition mask bias + 1/8 scale fused).
- The softmax denominator rides the ctx matmul: v is augmented with a
  ones column per head, so PSUM row 64 of ctx is the row sum.  Normalize =
  DVE reciprocal + Pool partition_broadcast + one strided fp16 multiply.
- probs are produced transposed [H, Lk, Lq] as fp16 and DMAd on the SWDGE
  queue; the host transposes back and upcasts while gathering.
"""

import numpy as np
from contextlib import ExitStack

import concourse.bass as bass
import concourse.mybir as mybir
import concourse.tile as tile
from concourse import bacc
from concourse.bass_utils import run_bass_kernel_spmd

F32 = mybir.dt.float32
F32R = mybir.dt.float32r
F16 = mybir.dt.float16
AF = mybir.ActivationFunctionType
ALU = mybir.AluOpType

HIDDEN = 768
HEADS = 12
D = 64
LQ = 512
LK = 1024
B = 8
P = 128
KT = HIDDEN // P          # 6 k-tiles of 128 (plus 1 aug row for q/k proj)
QT = LQ // P              # 4 l-tiles
MT = LK // P              # 8 m-tiles
LN_EPS = 1e-12


def build_nc():
    nc = bacc.Bacc("TRN2", target_bir_lowering=False, debug=False)

    # ---- DRAM I/O ------------------------------------------------------
    qT = nc.dram_tensor("qT", [HIDDEN, LQ], F16, kind="ExternalInput")
    kT = nc.dram_tensor("kT", [HIDDEN, LK], F16, kind="ExternalInput")
    vT = nc.dram_tensor("vT", [HIDDEN, LK], F16, kind="ExternalInput")
    wqT = nc.dram_tensor("wqT", [HIDDEN, HIDDEN], F16, kind="ExternalInput")
    wkT = nc.dram_tensor("wkT", [HIDDEN, HIDDEN], F16, kind="ExternalInput")
    bq2 = nc.dram_tensor("bq2", [P, KT], F32, kind="ExternalInput")
    bk2 = nc.dram_tensor("bk2", [P, KT], F32, kind="ExternalInput")
    wvT = nc.dram_tensor("wvT", [HIDDEN, HIDDEN], F16, kind="ExternalInput")
    woT = nc.dram_tensor("woT", [HIDDEN, HIDDEN], F16, kind="ExternalInput")
    q_res = nc.dram_tensor("q_res", [LQ, HIDDEN], F32, kind="ExternalInput")
    mask2 = nc.dram_tensor("mask2", [P, MT], F32, kind="ExternalInput")
    gamma = nc.dram_tensor("gamma", [HIDDEN], F32, kind="ExternalInput")
    beta = nc.dram_tensor("beta", [HIDDEN], F32, kind="ExternalInput")

    out = nc.dram_tensor("out", [LQ, HIDDEN], F32, kind="ExternalOutput")
    probsT = nc.dram_tensor("probsT", [HEADS, LK, LQ], F16, kind="ExternalOutput")

    with tile.TileContext(nc) as tc, ExitStack() as top:
        persist = top.enter_context(tc.tile_pool(name="persist", bufs=1))

        # persistent SBUF tensors
        qpT = [persist.tile([P, LQ], F16, name=f"qpT{j}") for j in range(KT)]
        kpT = [persist.tile([P, LK], F16, name=f"kpT{j}") for j in range(KT)]
        # v augmented with a per-head ones column: [m, 12*(64+1)]
        v_aug = [persist.tile([P, HEADS * (D + 1)], F16, name=f"vaug{m}")
                 for m in range(MT)]
        ctxT = [persist.tile([P, LQ], F16, name=f"ctxT{j}") for j in range(KT)]
        mask_sb = persist.tile([P, MT], F32, name="mask_sb")
        nc.sync.dma_start(out=mask_sb[:], in_=mask2[:])
        ones_f32 = persist.tile([P, HEADS, 1], F32, name="ones_f32")
        nc.vector.memset(ones_f32[:], 1.0)

        # ---- phase 1: QKV projections ---------------------------------
        with ExitStack() as proj:
            ppool = proj.enter_context(tc.tile_pool(name="proj_psum", bufs=2,
                                                    space="PSUM"))
            pq = proj.enter_context(tc.tile_pool(name="pq", bufs=1))
            wq_sb = [pq.tile([P, HIDDEN], F16, name=f"wq{j}") for j in range(KT)]
            qT_sb = [pq.tile([P, LQ], F16, name=f"qTs{j}") for j in range(KT)]
            bq_sb = pq.tile([P, KT], F32, name="bq_sb")
            bk_sb = pq.tile([P, KT], F32, name="bk_sb")
            nc.sync.dma_start(out=bq_sb[:], in_=bq2[:])
            nc.sync.dma_start(out=bk_sb[:], in_=bk2[:])
            for j in range(KT):
                nc.sync.dma_start(out=wq_sb[j][:], in_=wqT[j * P:(j + 1) * P, :])
                nc.sync.dma_start(out=qT_sb[j][:], in_=qT[j * P:(j + 1) * P, :])

            pk = proj.enter_context(tc.tile_pool(name="pk", bufs=1))
            wk_sb = [pk.tile([P, HIDDEN], F16, name=f"wk{j}") for j in range(KT)]
            kT_sb = [pk.tile([P, LK], F16, name=f"kTs{j}") for j in range(KT)]
            for j in range(KT):
                nc.sync.dma_start(out=wk_sb[j][:], in_=wkT[j * P:(j + 1) * P, :])
                nc.sync.dma_start(out=kT_sb[j][:], in_=kT[j * P:(j + 1) * P, :])

            pv = proj.enter_context(tc.tile_pool(name="pv", bufs=1))
            wv_sb = [pv.tile([P, HIDDEN], F16, name=f"wv{j}") for j in range(KT)]
            vT_sb = [pv.tile([P, LK], F16, name=f"vTs{j}") for j in range(KT)]

            for j in range(KT):
                nc.sync.dma_start(out=wv_sb[j][:], in_=wvT[j * P:(j + 1) * P, :])
                nc.sync.dma_start(out=vT_sb[j][:], in_=vT[j * P:(j + 1) * P, :])

            # q projection: qpT[io] [128, 512] = sum_j wq[j][:,io].T @ qT[j]
            for io in range(KT):
                ps = ppool.tile([P, LQ], F32, name="ps_p")
                for j in range(KT):
                    nc.tensor.matmul(ps[:], lhsT=wq_sb[j][:, io * P:(io + 1) * P],
                                     rhs=qT_sb[j][:], start=(j == 0),
                                     stop=(j == KT - 1))
                nc.vector.tensor_scalar_add(qpT[io][:], ps[:],
                                            bq_sb[:, io:io + 1])

            # k projection: kpT[io] [128, 1024], two 512-wide chunks
            for io in range(KT):
                for c in range(2):
                    cs = slice(c * 512, (c + 1) * 512)
                    ps = ppool.tile([P, 512], F32, name="ps_p")
                    for j in range(KT):
                        nc.tensor.matmul(ps[:], lhsT=wk_sb[j][:, io * P:(io + 1) * P],
                                         rhs=kT_sb[j][:, cs], start=(j == 0),
                                         stop=(j == KT - 1))
                    nc.vector.tensor_scalar_add(kpT[io][:, cs], ps[:],
                                                bk_sb[:, io:io + 1])

            # v projection -> v_aug strided per-head columns.
            # psum [m-tile 128, 384] covers heads 6c..6c+5 (6*64 outputs)
            va_view = [v.rearrange("p (h e) -> p h e", h=HEADS) for v in v_aug]
            for m in range(MT):
                for c in range(2):
                    ps = ppool.tile([P, 384], F32, name="ps_p")
                    for j in range(KT):
                        nc.tensor.matmul(
                            ps[:],
                            lhsT=vT_sb[j][:, m * P:(m + 1) * P],
                            rhs=wv_sb[j][:, c * 384:(c + 1) * 384],
                            start=(j == 0), stop=(j == KT - 1))
                    nc.vector.tensor_copy(
                        va_view[m][:, 6 * c:6 * c + 6, 0:D],
                        ps[:].rearrange("p (h e) -> p h e", h=6))
                # ones columns for the row-sum trick (walrus rejects
                # non-f32 memset here, so copy from an f32 staging tile)
                nc.vector.tensor_copy(va_view[m][:, :, D:D + 1], ones_f32[:])

        # output-projection weights: load during attention (sync queue idle)
        wop = top.enter_context(tc.tile_pool(name="wop", bufs=1))
        wo_sb = [wop.tile([P, HIDDEN], F16, name=f"wo{j}") for j in range(KT)]
        for j in range(KT):
            nc.sync.dma_start(out=wo_sb[j][:], in_=woT[j * P:(j + 1) * P, :])

        # ---- phase 2: attention ---------------------------------------
        attn = ExitStack()
        # scores for a head pair share one 2-bank psum tile -> one exp
        # instruction per m-tile with an exact per-partition mask bias
        sc_psum = attn.enter_context(tc.tile_pool(name="sc_psum", bufs=2,
                                                  space="PSUM"))
        cx_psum = attn.enter_context(tc.tile_pool(name="cx_psum", bufs=2,
                                                  space="PSUM"))
        pt_pool = attn.enter_context(tc.tile_pool(name="pt_pool", bufs=5))
        nrm_pool = attn.enter_context(tc.tile_pool(name="nrm_pool", bufs=3))

        pT_pairs = {}  # pair -> [128, MT*1024] tile (m-tile blocks of 1024:
                       #          [head even 512 | head odd 512])

        def emit_scores(pair):
            """scores + exp for heads (2*pair, 2*pair+1)."""
            j = pair
            pTp = pt_pool.tile([P, MT * 1024], F16, name="pT", tag="pT")
            pT_pairs[pair] = pTp
            for m in range(MT):
                ps = sc_psum.tile([P, 1024], F32, name="ps_s")
                for sub in range(2):
                    rows = slice(sub * D, (sub + 1) * D)
                    nc.tensor.matmul(ps[:, sub * 512:(sub + 1) * 512],
                                     lhsT=kpT[j][rows, m * P:(m + 1) * P],
                                     rhs=qpT[j][rows, :], start=True, stop=True)
                # p = exp(s/8 + mask): one eviction for both heads
                nc.scalar.activation(
                    pTp[:, m * 1024:(m + 1) * 1024], ps[:],
                    AF.Exp, bias=mask_sb[:, m:m + 1], scale=0.125)

        def emit_ctx(pair):
            """ctx matmul + softmax normalize + probs DMA for a done pair."""
            pTp = pT_pairs.pop(pair)
            cps_pair = []
            for sub in range(2):
                h = 2 * pair + sub
                cps = cx_psum.tile([D + 1, LQ], F32, name="ps_c")
                cps_pair.append(cps)
                for m in range(MT):
                    nc.tensor.matmul(
                        cps[:],
                        lhsT=v_aug[m][:, h * (D + 1):(h + 1) * (D + 1)],
                        rhs=pTp[:, m * 1024 + sub * 512:
                                m * 1024 + (sub + 1) * 512],
                        start=(m == 0), stop=(m == MT - 1))
                # evict ctx rows early to free psum (DVE, casts to fp16)
                prow = slice((h % 2) * D, (h % 2) * D + D)
                nc.vector.tensor_copy(ctxT[h // 2][prow, :], cps[0:D, :])
            # pair reciprocal of rowsums (psum row 64)
            recips = []
            for sub in range(2):
                recip = nrm_pool.tile([1, LQ], F16, name="recip")
                with nc.allow_low_precision(reason="fp16 softmax recip"):
                    nc.vector.reciprocal(recip[:], cps_pair[sub][D:D + 1, :])
                recips.append(recip)
            for sub in range(2):
                h = 2 * pair + sub
                prow = slice((h % 2) * D, (h % 2) * D + D)
                # broadcast recip across partitions (Pool)
                recip_b = nrm_pool.tile([P, LQ], F16, name="recip_b")
                nc.gpsimd.partition_broadcast(recip_b[:], recips[sub][:])
                # normalize ctx rows and probs (DVE, fp16)
                nc.vector.tensor_mul(ctxT[h // 2][prow, :],
                                     ctxT[h // 2][prow, :], recip_b[prow, :])
                ph_v = bass.AP(tensor=pTp.tensor,
                               offset=pTp.offset + sub * 512,
                               ap=[list(pTp.ap[0]), [1024, MT], [1, 512]])
                rb_rep = bass.AP(tensor=recip_b.tensor, offset=recip_b.offset,
                                 ap=[list(recip_b.ap[0]), [0, MT],
                                     list(recip_b.ap[1])])
                nc.vector.tensor_mul(ph_v, ph_v, rb_rep)
                # probs out on the SWDGE queue (keeps sync HWDGE FIFO clear)
                dram = probsT[h].rearrange("(m p) l -> p m l", p=P)
                nc.gpsimd.dma_start(out=dram, in_=ph_v)

        for pair in range(6):
            emit_scores(pair)
            if pair >= 2:
                emit_ctx(pair - 2)
        emit_ctx(4)
        emit_ctx(5)

        # tail pools open inside the attention scope: their PSUM banks and
        # SBUF are disjoint from the attention pools, so the output dense
        # can start while the last softmax chains drain
        late = attn.enter_context(tc.tile_pool(name="late", bufs=1))
        qres_sb = [late.tile([P, HIDDEN], F32, name=f"qres{t}") for t in range(QT)]
        for t in range(QT):
            nc.sync.dma_start(out=qres_sb[t][:], in_=q_res[t * P:(t + 1) * P, :])
        gamma_sb = late.tile([P, HIDDEN], F32, name="gamma_sb")
        beta_sb = late.tile([P, HIDDEN], F32, name="beta_sb")
        nc.gpsimd.dma_start(out=gamma_sb[:], in_=gamma.ap().partition_broadcast(P))
        nc.gpsimd.dma_start(out=beta_sb[:], in_=beta.ap().partition_broadcast(P))

        # ---- phase 3: output dense + residual + LayerNorm -------------
        o_psum = attn.enter_context(tc.tile_pool(name="o_psum", bufs=2,
                                                 space="PSUM"))
        h_pool = attn.enter_context(tc.tile_pool(name="h_pool", bufs=2))
        st_pool = attn.enter_context(tc.tile_pool(name="st_pool", bufs=4))
        eps_sb = late.tile([P, 1], F32, name="eps_sb")
        nc.vector.memset(eps_sb[:], LN_EPS)

        for t in range(QT):
            ls = slice(t * P, (t + 1) * P)
            h_sb = h_pool.tile([P, HIDDEN], F32, name="h_sb")
            for c in range(2):
                cs = slice(c * 384, (c + 1) * 384)
                ps = o_psum.tile([P, 384], F32, name="ps_o")
                for j in range(KT):
                    nc.tensor.matmul(ps[:], lhsT=ctxT[j][:, ls],
                                     rhs=wo_sb[j][:, cs],
                                     start=(j == 0), stop=(j == KT - 1))
                # h = ctx@WoT + (query + bo + bv@WoT)
                nc.vector.scalar_tensor_tensor(
                    out=h_sb[:, cs], in0=ps[:], scalar=1.0,
                    in1=qres_sb[t][:, cs], op0=ALU.mult, op1=ALU.add)
            # LayerNorm over free dim (768) via bn_stats subgroups of 256
            stats = st_pool.tile([P, 3, 6], F32, name="stats")
            hv = h_sb.rearrange("p (s d) -> p s d", s=3)
            for s in range(3):
                nc.vector.bn_stats(out=stats[:, s, :], in_=hv[:, s, :])
            mv = st_pool.tile([P, 2], F32, name="mv")
            nc.vector.bn_aggr(out=mv[:], in_=stats[:])
            # rstd = 1/sqrt(var + eps)
            rstd = st_pool.tile([P, 1], F32, name="rstd")
            nc.scalar.activation(rstd[:], mv[:, 1:2], AF.Sqrt, bias=eps_sb[:])
            nc.vector.reciprocal(rstd[:], rstd[:])
            # out = (h - mu) * rstd * gamma + beta
            nc.vector.tensor_scalar_sub(h_sb[:], h_sb[:], mv[:, 0:1])
            o_sb = h_pool.tile([P, HIDDEN], F32, name="o_sb")
            nc.vector.scalar_tensor_tensor(
                out=o_sb[:], in0=h_sb[:], scalar=rstd[:],
                in1=gamma_sb[:], op0=ALU.mult, op1=ALU.mult)
            nc.vector.tensor_add(o_sb[:], o_sb[:], beta_sb[:])
            nc.sync.dma_start(out=out[ls, :], in_=o_sb[:])

        attn.close()

    nc.compile()
    return nc


_NC_CACHE = {}


def _get_nc():
    if "nc" not in _NC_CACHE:
        _NC_CACHE["nc"] = build_nc()
    return _NC_CACHE["nc"]


def _prep_core(query_b, key_b, value_b, mask_b, WqT, WkT, WvT, WoT,
               q_res_b, bq2, bk2, gamma, beta):
    qT = np.ascontiguousarray(query_b.T.astype(np.float16))
    kT = np.ascontiguousarray(key_b.T.astype(np.float16))
    vT = np.ascontiguousarray(value_b.T.astype(np.float16))
    mask2 = np.ascontiguousarray(mask_b.reshape(MT, P).T)
    return {
        "qT": qT, "kT": kT, "vT": vT,
        "wqT": WqT, "wkT": WkT, "wvT": WvT, "woT": WoT,
        "q_res": q_res_b, "bq2": bq2, "bk2": bk2, "mask2": mask2,
        "gamma": gamma, "beta": beta,
    }


def _run(inputs, trace=False):
    query = np.asarray(inputs["query"], np.float32)
    key = np.asarray(inputs["key"], np.float32)
    value = np.asarray(inputs["value"], np.float32)
    mask = np.asarray(inputs["attention_mask"], np.float32)
    Wq = np.asarray(inputs["Wq"], np.float32)
    bq = np.asarray(inputs["bq"], np.float32)
    Wk = np.asarray(inputs["Wk"], np.float32)
    bk = np.asarray(inputs["bk"], np.float32)
    Wv = np.asarray(inputs["Wv"], np.float32)
    bv = np.asarray(inputs["bv"], np.float32)
    Wo = np.asarray(inputs["Wo"], np.float32)
    bo = np.asarray(inputs["bo"], np.float32)
    gamma = np.asarray(inputs["ln_gamma"], np.float32)
    beta = np.asarray(inputs["ln_beta"], np.float32)

    WqT = np.ascontiguousarray(Wq.T.astype(np.float16))
    WkT = np.ascontiguousarray(Wk.T.astype(np.float16))
    WvT = np.ascontiguousarray(Wv.T.astype(np.float16))
    WoT = np.ascontiguousarray(Wo.T.astype(np.float16))
    bq2 = np.ascontiguousarray(bq.reshape(KT, P).T)
    bk2 = np.ascontiguousarray(bk.reshape(KT, P).T)
    res_const = (bo + bv @ Wo.T)[None, :]

    nc = _get_nc()
    in_maps = []
    for b in range(B):
        q_res_b = np.ascontiguousarray(query[b] + res_const)
        in_maps.append(_prep_core(query[b], key[b], value[b], mask[b, 0, 0],
                                  WqT, WkT, WvT, WoT, q_res_b, bq2, bk2,
                                  gamma, beta))
    res = run_bass_kernel_spmd(nc, in_maps, list(range(B)), trace=trace)
    out = np.stack([res.results[c]["out"] for c in range(B)])
    probs = np.stack([res.results[c]["probsT"].astype(np.float32).transpose(0, 2, 1)
                      for c in range(B)])
    return out, probs, res


def kernel(**inputs):
    out, probs, _ = _run(inputs)
    return out, probs
